# revision 1
# baseline (speedup 1.0000x reference)
"""CTC loss (mean, zero_infinity) on 8 Trainium2 NeuronCores.

Data-parallel over batch: 4 samples/core. Per core, a prob-domain CTC
forward DP in a rotated label coordinate system:

  - labels rotated so the final answer always sits at partition 127
  - absorbing phase (t >= input_len): pb=1, pl=0 folds logaddexp(alpha[end],
    alpha[end-1]) into blank[tl] and freezes the state
  - windowed rescaling every R steps: zero outside the exact feasibility
    cone, rescale by the cone colsum recentred at CENTER=1e33 (all exactly
    zero-contribution states, so no approximation beyond f32)

Layout: state tiles BL/LA are (128 part = rotated label idx, 4 free =
sample). Per step: 3 PE matmuls produce SH=shift(LA) and C1=BL+SH in PSUM;
5 DVE ops apply the probability tiles. Probability tiles PL/QL/PB are
(128, T, 4) f32 in SBUF, produced by a one-hot fp32 gather matmul from the
transposed log_probs plus Exp on the scalar engine.
"""

import numpy as np

import concourse.bass as bass
import concourse.bacc as bacc
import concourse.tile as tile
from concourse import mybir
from concourse.bass_utils import run_bass_kernel_spmd

F32 = mybir.dt.float32
I32 = mybir.dt.int32
AF = mybir.ActivationFunctionType
OP = mybir.AluOpType

T_FULL = 1024
V = 512
L = 128
NB = 4            # samples per core
NCORES = 8
R = 8             # rescale interval
CENTER = 1e33
NEG = -1e30


def build_nc(T=T_FULL, debug_dump=False, a_on_pool=False, use_psumB=True):
    nc = bacc.Bacc("TRN2", target_bir_lowering=False, debug=False, num_devices=NCORES)

    lpT = nc.dram_tensor("lpT", [NB, V, T], F32, kind="ExternalInput")
    lpb = nc.dram_tensor("lpb", [NB, T], F32, kind="ExternalInput")
    thr = nc.dram_tensor("thr", [NB, T], F32, kind="ExternalInput")
    tmk = nc.dram_tensor("tmk", [NB, T], F32, kind="ExternalInput")
    tgtrot = nc.dram_tensor("tgtrot", [NB, L], F32, kind="ExternalInput")
    skiprot = nc.dram_tensor("skiprot", [L, NB], F32, kind="ExternalInput")
    rows = nc.dram_tensor("rows", [4, 512], F32, kind="ExternalInput")
    outd = nc.dram_tensor("out", [2, NB], F32, kind="ExternalOutput")
    outw = nc.dram_tensor("logw", [1, 512], F32, kind="ExternalOutput")
    if debug_dump:
        dbgPL = nc.dram_tensor("dbgPL", [128, 16, NB], F32, kind="ExternalOutput")
        dbgPB = nc.dram_tensor("dbgPB", [128, 16, NB], F32, kind="ExternalOutput")
        dbgQL = nc.dram_tensor("dbgQL", [128, 16, NB], F32, kind="ExternalOutput")
        dbgI = nc.dram_tensor("dbgI", [128, 2 * NB], F32, kind="ExternalOutput")
        dbgS = nc.dram_tensor("dbgS", [128, 8 * NB], F32, kind="ExternalOutput")
        dbgM = nc.dram_tensor("dbgM", [128, 16], F32, kind="ExternalOutput")

    n_tc = T // 512 if T >= 512 else 1   # t-chunks for gather
    TC = min(T, 512)
    n_rs = (T - 1) // R                  # number of rescales (t = R..n_rs*R)

    with tile.TileContext(nc) as tc:
        with tc.tile_pool(name="const", bufs=1) as const, \
             tc.tile_pool(name="bigp", bufs=1) as bigp, \
             tc.tile_pool(name="maskp", bufs=1) as maskp:

            # ---------- constants ----------
            ones_col = const.tile([128, 1], F32)
            nc.vector.memset(ones_col, 1.0)
            ones_row = const.tile([1, 128], F32)
            nc.vector.memset(ones_row, 1.0)

            # io_f[k, m] = m ; io_p[k, m] = k  -> SUB = (m - k == 1), ID = (== 0)
            io_f_i = const.tile([128, 128], I32)
            nc.gpsimd.iota(io_f_i, pattern=[[1, 128]], base=0, channel_multiplier=0)
            io_p_i = const.tile([128, 128], I32)
            nc.gpsimd.iota(io_p_i, pattern=[[0, 128]], base=0, channel_multiplier=1)
            io_f = const.tile([128, 128], F32)
            nc.vector.tensor_copy(io_f, io_f_i)
            io_p = const.tile([128, 128], F32)
            nc.vector.tensor_copy(io_p, io_p_i)
            dmk = const.tile([128, 128], F32)
            nc.vector.tensor_sub(dmk, io_f, io_p)       # m - k
            SUB = const.tile([128, 128], F32)
            nc.vector.tensor_scalar(SUB, dmk, 1.0, None, OP.is_equal)
            ID = const.tile([128, 128], F32)
            nc.vector.tensor_scalar(ID, dmk, 0.0, None, OP.is_equal)

            # iota_k per v-chunk: (128,1) f32 with value k + 128*vc
            iota_k = []
            for vc in range(4):
                ik_i = const.tile([128, 1], I32, tag=f"ik{vc}i")
                nc.gpsimd.iota(ik_i, pattern=[[0, 1]], base=128 * vc, channel_multiplier=1)
                ik = const.tile([128, 1], F32, tag=f"ik{vc}")
                nc.vector.tensor_copy(ik, ik_i)
                iota_k.append(ik)

            # ---------- small input loads ----------
            # per-sample rows each in their own partition-0 tile (matmul rhs
            # base-partition constraint)
            lpb_b, thr_b, tmk_b, tgt_b = [], [], [], []
            for b in range(NB):
                tb = const.tile([1, T], F32, tag=f"lpb{b}")
                nc.sync.dma_start(out=tb, in_=lpb[b:b + 1, :])
                lpb_b.append(tb)
                tb = const.tile([1, T], F32, tag=f"thr{b}")
                nc.sync.dma_start(out=tb, in_=thr[b:b + 1, :])
                thr_b.append(tb)
                tb = const.tile([1, T], F32, tag=f"tmk{b}")
                nc.sync.dma_start(out=tb, in_=tmk[b:b + 1, :])
                tmk_b.append(tb)
                tb = const.tile([1, L], F32, tag=f"tgt{b}")
                nc.sync.dma_start(out=tb, in_=tgtrot[b:b + 1, :])
                tgt_b.append(tb)
            skpS = const.tile([L, NB], F32)
            nc.sync.dma_start(out=skpS, in_=skiprot[:, :])
            rows_b = []
            for i in range(4):
                tb = const.tile([1, 512], F32, tag=f"rows{i}")
                nc.sync.dma_start(out=tb, in_=rows[i:i + 1, :])
                rows_b.append(tb)

            # ---------- rescale-window masks ----------
            # sblf[i, (r,b)] = 2*i + soff_b   (soff = 2*tl - 254)
            mbl = maskp.tile([128, 512], F32)
            mla = maskp.tile([128, 512], F32)
            initBC = maskp.tile([128, NB], F32)
            initLC = maskp.tile([128, NB], F32)
            LOGW = maskp.tile([1, 512], F32)
            nc.vector.memset(LOGW, 0.0)

            with tc.tile_pool(name="psm", bufs=2, space="PSUM") as psm, \
                 tc.tile_pool(name="mtmp", bufs=2) as mtmp:
                i2_i = mtmp.tile([128, 512], I32)
                nc.gpsimd.iota(i2_i, pattern=[[0, 512]], base=0, channel_multiplier=2)
                i2f = mtmp.tile([128, 512], F32, tag="i2f")
                nc.vector.tensor_copy(i2f, i2_i)
                pm = psm.tile([128, 512], F32, tag="pm")
                nc.tensor.matmul(pm, ones_row, rows_b[0], start=True, stop=True)
                sblf = mtmp.tile([128, 512], F32, tag="sblf")
                nc.vector.tensor_add(sblf, i2f, pm)
                pm2 = psm.tile([128, 512], F32, tag="pm")
                nc.tensor.matmul(pm2, ones_row, rows_b[1], start=True, stop=True)
                c1 = mtmp.tile([128, 512], F32, tag="c1")
                nc.vector.tensor_tensor(c1, sblf, pm2, OP.is_ge)
                pm3 = psm.tile([128, 512], F32, tag="pm")
                nc.tensor.matmul(pm3, ones_row, rows_b[2], start=True, stop=True)
                c2 = mtmp.tile([128, 512], F32, tag="c2")
                nc.vector.tensor_tensor(c2, sblf, pm3, OP.is_le)
                nc.vector.tensor_mul(mbl, c1, c2)
                # lab window: s_la = s_bl + 1; need s_la in [lo-1, hi]
                # i.e. s_bl >= lo - 2  and  s_bl <= hi - 1
                # use sblf2 = s_bl + 2:  (sblf2 >= lo) and (sblf2 <= hi + 1)
                pm4 = psm.tile([128, 512], F32, tag="pm4")
                nc.tensor.matmul(pm4, ones_row, rows_b[3], start=True, stop=True)
                sblf2 = mtmp.tile([128, 512], F32, tag="sblf2")
                nc.vector.tensor_scalar(sblf2, sblf, 2.0, None, OP.add)
                c1l = mtmp.tile([128, 512], F32, tag="c1l")
                nc.vector.tensor_tensor(c1l, sblf2, pm2, OP.is_ge)
                c2l = mtmp.tile([128, 512], F32, tag="c2l")
                nc.vector.tensor_tensor(c2l, sblf2, pm4, OP.is_le)
                nc.vector.tensor_mul(mla, c1l, c2l)
                # init: blank[0] and lab[1] both live at i = r  (s_bl == 0)
                initB = mtmp.tile([128, NB], F32, tag="ib")
                nc.vector.tensor_scalar(initB, sblf[:, 0:NB], 0.0, None, OP.is_equal)
                nc.vector.tensor_scalar(initBC, initB, CENTER, None, OP.mult)
                nc.vector.tensor_scalar(initLC, initB, CENTER, None, OP.mult)

            # ---------- probability tiles ----------
            PL = bigp.tile([128, T, NB], F32)
            QL = bigp.tile([128, T, NB], F32)
            PB = bigp.tile([128, T, NB], F32)

            # lpb masked for absorbing phase: pb=1 <=> lp=0
            lpbm_b = []
            for b in range(NB):
                tb = const.tile([1, T], F32, tag=f"lpbm{b}")
                nc.vector.tensor_mul(tb, lpb_b[b], tmk_b[b])
                lpbm_b.append(tb)

            with tc.tile_pool(name="psg", bufs=2, space="PSUM") as psg, \
                 tc.tile_pool(name="psT", bufs=2, space="PSUM") as psT, \
                 tc.tile_pool(name="stage", bufs=3) as stage, \
                 tc.tile_pool(name="ohp", bufs=2) as ohp:
                for b in range(NB):
                    # one-hot: oh_vc[k, m] = (tgt[b, m] == k + 128*vc)
                    pT = psT.tile([128, L], F32, tag="pT")
                    nc.tensor.matmul(pT, ones_row, tgt_b[b], start=True, stop=True)
                    ohs = []
                    for vc in range(4):
                        oh = ohp.tile([128, L], F32, tag=f"oh{vc}")
                        nc.vector.tensor_scalar(oh, pT, iota_k[vc], None, OP.is_equal,
                                                OP.bypass)
                        ohs.append(oh)
                    for tci in range(n_tc):
                        pg = psg.tile([128, TC], F32, tag="pg")
                        for vc in range(4):
                            st = stage.tile([128, TC], F32, tag="st")
                            nc.sync.dma_start(
                                out=st,
                                in_=lpT[b, 128 * vc:128 * (vc + 1),
                                        TC * tci:TC * (tci + 1)])
                            nc.tensor.matmul(pg, ohs[vc], st,
                                             start=(vc == 0), stop=False)
                        # absorbing mask add: + (-1e30 * [t >= il]) broadcast over i
                        nc.tensor.matmul(pg, ones_row,
                                         thr_b[b][:, TC * tci:TC * (tci + 1)],
                                         start=False, stop=True)
                        nc.scalar.activation(PL[:, TC * tci:TC * (tci + 1), b], pg,
                                             AF.Exp)
                        # blank probs: bcast lpbm row then exp
                        pgb = psg.tile([128, TC], F32, tag="pg")
                        nc.tensor.matmul(pgb, ones_row,
                                         lpbm_b[b][:, TC * tci:TC * (tci + 1)],
                                         start=True, stop=True)
                        nc.scalar.activation(PB[:, TC * tci:TC * (tci + 1), b], pgb,
                                             AF.Exp)
                    # QL = PL * skip (per-partition scalar per sample)
                    nc.vector.tensor_scalar(QL[:, :, b], PL[:, :, b],
                                            skpS[:, b:b + 1], None, OP.mult)

            # ---------- DP ----------
            with tc.tile_pool(name="pstep", bufs=2, space="PSUM") as pstep, \
                 tc.tile_pool(name="psrs", bufs=2, space="PSUM") as psrs, \
                 tc.tile_pool(name="work", bufs=3) as work:

                BL = work.tile([128, NB], F32, tag="BL")
                nc.vector.tensor_mul(BL, initBC, PB[:, 0, :])
                LA = work.tile([128, NB], F32, tag="LA")
                nc.vector.tensor_mul(LA, initLC, PL[:, 0, :])
                if debug_dump:
                    nc.sync.dma_start(out=dbgPL[:, :, :], in_=PL[:, 0:16, :])
                    nc.sync.dma_start(out=dbgPB[:, :, :], in_=PB[:, 0:16, :])
                    nc.sync.dma_start(out=dbgQL[:, :, :], in_=QL[:, 0:16, :])
                    nc.sync.dma_start(out=dbgI[:, 0:NB], in_=initBC)
                    nc.sync.dma_start(out=dbgI[:, NB:2 * NB], in_=initLC)
                    nc.sync.dma_start(out=dbgS[:, 0:NB], in_=BL)
                    nc.sync.dma_start(out=dbgS[:, NB:2 * NB], in_=LA)
                    nc.sync.dma_start(out=dbgM[:, 0:8], in_=mbl[:, 0:8])
                    nc.sync.dma_start(out=dbgM[:, 8:16], in_=mla[:, 0:8])

                for t in range(1, T):
                    pA = pstep.tile([128, NB], F32, tag="psA")
                    nc.tensor.matmul(pA, SUB, LA, start=True, stop=True)
                    if use_psumB:
                        pB = pstep.tile([128, NB], F32, tag="psB")
                        nc.tensor.matmul(pB, SUB, LA, start=True, stop=False)
                        nc.tensor.matmul(pB, ID, BL, start=False, stop=True)
                    A1 = work.tile([128, NB], F32, tag="A1")
                    if a_on_pool:
                        nc.gpsimd.tensor_add(A1, LA, BL)
                    else:
                        nc.vector.tensor_add(A1, LA, BL)
                    B1 = work.tile([128, NB], F32, tag="B1")
                    nc.vector.tensor_tensor(B1, A1, PL[:, t, :], OP.mult)
                    B2v = work.tile([128, NB], F32, tag="B2v")
                    nc.vector.tensor_tensor(B2v, pA, QL[:, t, :], OP.mult)
                    LAn = work.tile([128, NB], F32, tag="LA")
                    nc.vector.tensor_add(LAn, B1, B2v)
                    BLn = work.tile([128, NB], F32, tag="BL")
                    if use_psumB:
                        nc.vector.tensor_tensor(BLn, pB, PB[:, t, :], OP.mult)
                    else:
                        C1 = work.tile([128, NB], F32, tag="C1")
                        nc.vector.tensor_add(C1, BL, pA)
                        nc.vector.tensor_tensor(BLn, C1, PB[:, t, :], OP.mult)
                    LA, BL = LAn, BLn
                    if debug_dump and t in (1, 2):
                        nc.sync.dma_start(out=dbgS[:, (2 * t) * NB:(2 * t + 1) * NB], in_=BL)
                        nc.sync.dma_start(out=dbgS[:, (2 * t + 1) * NB:(2 * t + 2) * NB], in_=LA)

                    if t % R == 0 and t // R <= n_rs:
                        ri = t // R - 1
                        BLm = work.tile([128, NB], F32, tag="BLm")
                        nc.vector.tensor_tensor(BLm, BL,
                                                mbl[:, NB * ri:NB * (ri + 1)], OP.mult)
                        LAm = work.tile([128, NB], F32, tag="LAm")
                        nc.vector.tensor_tensor(LAm, LA,
                                                mla[:, NB * ri:NB * (ri + 1)], OP.mult)
                        pS = psrs.tile([1, NB], F32, tag="psS")
                        nc.tensor.matmul(pS, ones_col, BLm, start=True, stop=False)
                        nc.tensor.matmul(pS, ones_col, LAm, start=False, stop=True)
                        wrec = work.tile([1, NB], F32, tag="wrec")
                        nc.vector.reciprocal(wrec, pS)
                        wrecC = work.tile([1, NB], F32, tag="wrecC")
                        nc.vector.tensor_scalar(wrecC, wrec, CENTER, None, OP.mult)
                        # store raw w; host takes logs in f64 (ACT Ln is
                        # far too inaccurate: ~1 nat bias)
                        nc.scalar.copy(LOGW[0:1, NB * ri:NB * (ri + 1)], pS)
                        pR = psrs.tile([128, NB], F32, tag="psR")
                        nc.tensor.matmul(pR, ones_row, wrecC, start=True, stop=True)
                        BL2 = work.tile([128, NB], F32, tag="BL")
                        nc.vector.tensor_tensor(BL2, BLm, pR, OP.mult)
                        LA2 = work.tile([128, NB], F32, tag="LA")
                        nc.vector.tensor_tensor(LA2, LAm, pR, OP.mult)
                        LA, BL = LA2, BL2

                # ---------- output ----------
                nc.sync.dma_start(out=outw[0:1, :], in_=LOGW)
                nc.sync.dma_start(out=outd[1:2, :], in_=BL[127:128, :])

    nc.compile()
    return nc


def host_prep(log_probs, targets, input_lengths, target_lengths, T=T_FULL):
    """Build the per-core input maps (host-side sharding + index prep)."""
    log_probs = np.asarray(log_probs, np.float32)
    targets = np.asarray(targets).astype(np.int64)
    il = np.asarray(input_lengths).astype(np.int64)
    tl = np.asarray(target_lengths).astype(np.int64)
    B = log_probs.shape[0]
    in_maps = []
    t_ar = np.arange(T)
    for c in range(NCORES):
        s = slice(c * NB, (c + 1) * NB)
        lp = log_probs[s, :T]
        ilc, tlc = il[s], tl[s]
        tg = targets[s]
        lpT = np.ascontiguousarray(np.transpose(lp, (0, 2, 1)))      # (NB, V, T)
        lpb = np.ascontiguousarray(lp[:, :, 0])                      # (NB, T)
        absorb = t_ar[None, :] >= ilc[:, None]
        thr = np.where(absorb, np.float32(NEG), np.float32(0.0)).astype(np.float32)
        tmk = np.where(absorb, np.float32(0.0), np.float32(1.0)).astype(np.float32)
        rot = 127 - tlc                                              # (NB,)
        tgtrot = np.full((NB, L), -1.0, np.float32)
        skiprot = np.zeros((L, NB), np.float32)
        for b in range(NB):
            r0 = rot[b]
            n = tlc[b]
            tgtrot[b, r0:r0 + n] = tg[b, :n].astype(np.float32)
            if n > 1:
                sk = (tg[b, 1:n] != tg[b, :n - 1]).astype(np.float32)
                skiprot[r0 + 1:r0 + n, b] = sk
        # rescale rows: col 4*ri + b ; t_r = R*(ri+1)
        n_ri = 128
        ri = np.arange(n_ri)
        t_r = R * (ri + 1)
        lo = 2 * tlc[None, :] - 2 * np.maximum(ilc[None, :] - t_r[:, None], 0)
        hi = np.minimum(2 * t_r[:, None] + 1, 2 * tlc[None, :])
        soff = np.broadcast_to((2 * tlc - 254)[None, :], (n_ri, NB))
        rows = np.zeros((4, 512), np.float32)
        rows[0, :] = soff.reshape(-1)[:512]
        rows[1, :] = lo.reshape(-1)[:512]
        rows[2, :] = hi.reshape(-1)[:512]
        rows[3, :] = (hi + 1).reshape(-1)[:512]
        in_maps.append({
            "lpT": lpT, "lpb": lpb, "thr": thr, "tmk": tmk,
            "tgtrot": tgtrot, "skiprot": skiprot, "rows": rows,
        })
    return in_maps


_NC_CACHE = {}


def _get_nc(T=T_FULL):
    if T not in _NC_CACHE:
        _NC_CACHE[T] = build_nc(T)
    return _NC_CACHE[T]


def finish(results, target_lengths, T=T_FULL):
    tl = np.asarray(target_lengths).astype(np.int64)
    n_rs = (T - 1) // R
    pers = []
    for c in range(NCORES):
        out = results[c]["out"]
        fin = out[1].astype(np.float64)
        wv = results[c]["logw"][0].astype(np.float64).reshape(128, NB)[:n_rs]
        logs = np.log(np.maximum(wv, 1e-300)).sum(0)
        tlc = tl[c * NB:(c + 1) * NB].astype(np.float64)
        ll = np.log(np.maximum(fin, 1e-300)) + logs - (n_rs + 1) * np.log(CENTER)
        per = -ll / tlc
        pers.append(per)
    return np.float32(np.mean(np.concatenate(pers)))


def kernel(log_probs, targets, input_lengths, target_lengths):
    nc = _get_nc()
    in_maps = host_prep(log_probs, targets, input_lengths, target_lengths)
    res = run_bass_kernel_spmd(nc, in_maps, core_ids=list(range(NCORES)))
    return finish(res.results, target_lengths)



# revision 9
# speedup vs baseline: 1.6507x; 1.6507x over previous
"""CTC loss (mean, zero_infinity) on 8 Trainium2 NeuronCores.

Data-parallel over batch: 4 samples/core. Per core, a prob-domain CTC
forward DP in a rotated label coordinate system (final answer at
partition 127), with TWO time steps fused per device-loop iteration:

  - 2-step coefficient tiles are precomputed in bulk (wide DVE ops over
    all 512 step-pairs at once); each serial super-step then needs only
    3 tiny PE matmuls (identity/shift/shift2 of the state into PSUM)
    plus one fused tensor-tensor multiply and one reduce on DVE.
  - absorbing phase (t >= input_len): pb=1, pl=0 folds
    logaddexp(alpha[end], alpha[end-1]) into blank[tl] and freezes the
    state; an extra absorbing pad step at t=1024 rounds the 1023 DP
    steps up to 512 exact pairs (and folds the il==1024 edge case).
  - windowed rescaling every R=16 steps (8 super-steps): zero outside
    the exact feasibility cone, rescale by the cone colsum recentred at
    CENTER=1e33.

State tile S is (128 part = rotated label idx, [fam(2: LA,BL), sample(4)]).
Coefficient tiles CLCB are (128, 512 pairs, fam 2, sample 4, term 5) where
term indexes the basis [S, sh(S), sh2(LA)] expansion of the 2-step map:
  LA2 = c0·LA + c1·BL + c2·shLA + c3·shBL + c4·sh2LA
  BL2 =         d1·BL + d2·shLA + d3·shBL + d4·sh2LA
  c0 = PL2·PL1            d1 = PB2·PB1
  c1 = PL2·(PL1+PB1)      d2 = PB2·(PB1+shPL1)
  c2 = PL2·(QL1+PB1)+QL2·shPL1
  c3 = QL2·shPL1          d3 = PB2·shPL1
  c4 = QL2·shQL1          d4 = PB2·shQL1
(1 = odd step t=2tau-1, 2 = even step t=2tau, sh = value at label s-1.)
"""

import numpy as np

import concourse.bass as bass
import concourse.bacc as bacc
import concourse.tile as tile
from concourse import mybir
from concourse.bass_utils import run_bass_kernel_spmd

F32 = mybir.dt.float32
I32 = mybir.dt.int32
AF = mybir.ActivationFunctionType
OP = mybir.AluOpType

T_FULL = 1024
V = 512
L = 128
NB = 4            # samples per core
NCORES = 8
R = 16            # rescale interval (in original t-steps); 8 super-steps
CENTER = 1e28
NEG = -1e30

TP = T_FULL + 1   # prob tiles padded with absorbing step at t=1024
NTAU = T_FULL // 2  # 512 super-steps covering t=1..1024


def build_nc(debug_dump=False):
    T = T_FULL
    nc = bacc.Bacc("TRN2", target_bir_lowering=False, debug=False, num_devices=NCORES)

    lpT = nc.dram_tensor("lpT", [NB, V, T], F32, kind="ExternalInput")
    lpbm = nc.dram_tensor("lpbm", [NB, T], F32, kind="ExternalInput")
    thr = nc.dram_tensor("thr", [NB, T], F32, kind="ExternalInput")
    tgtrot = nc.dram_tensor("tgtrot", [NB, L], F32, kind="ExternalInput")
    skiprot = nc.dram_tensor("skiprot", [L, NB], F32, kind="ExternalInput")
    valrot = nc.dram_tensor("valrot", [L, NB], F32, kind="ExternalInput")
    rows = nc.dram_tensor("rows", [4, 512], F32, kind="ExternalInput")
    outd = nc.dram_tensor("out", [2, NB], F32, kind="ExternalOutput")
    outw = nc.dram_tensor("logw", [1, 512], F32, kind="ExternalOutput")

    n_tc = 2
    TC = 512
    n_rs = (T - 1) // R                  # 63 rescales (t = 16..1008)

    with tile.TileContext(nc) as tc:
        with tc.tile_pool(name="const", bufs=1) as const, \
             tc.tile_pool(name="bigp", bufs=1) as bigp, \
             tc.tile_pool(name="coefp", bufs=1) as coefp, \
             tc.tile_pool(name="maskp", bufs=1) as maskp:

            # ---------- constants ----------
            ones_col = const.tile([128, 1], F32)
            nc.vector.memset(ones_col, 1.0)
            ones_row = const.tile([1, 128], F32)
            nc.vector.memset(ones_row, 1.0)

            io_f_i = const.tile([128, 128], I32)
            nc.gpsimd.iota(io_f_i, pattern=[[1, 128]], base=0, channel_multiplier=0)
            io_p_i = const.tile([128, 128], I32)
            nc.gpsimd.iota(io_p_i, pattern=[[0, 128]], base=0, channel_multiplier=1)
            io_f = const.tile([128, 128], F32)
            nc.vector.tensor_copy(io_f, io_f_i)
            io_p = const.tile([128, 128], F32)
            nc.vector.tensor_copy(io_p, io_p_i)
            dmk = const.tile([128, 128], F32)
            nc.vector.tensor_sub(dmk, io_f, io_p)       # m - k
            SUB = const.tile([128, 128], F32)
            nc.vector.tensor_scalar(SUB, dmk, 1.0, None, OP.is_equal)
            SUB2 = const.tile([128, 128], F32)
            nc.vector.tensor_scalar(SUB2, dmk, 2.0, None, OP.is_equal)
            ID = const.tile([128, 128], F32)
            nc.vector.tensor_scalar(ID, dmk, 0.0, None, OP.is_equal)

            iota_k = []
            for vc in range(4):
                ik_i = const.tile([128, 1], I32, tag=f"ik{vc}i")
                nc.gpsimd.iota(ik_i, pattern=[[0, 1]], base=128 * vc, channel_multiplier=1)
                ik = const.tile([128, 1], F32, tag=f"ik{vc}")
                nc.vector.tensor_copy(ik, ik_i)
                iota_k.append(ik)

            # ---------- rescale-window masks ----------
            mbl = maskp.tile([128, 512], F32)
            mla = maskp.tile([128, 512], F32)
            initBC = maskp.tile([128, NB], F32)
            initLC = maskp.tile([128, NB], F32)
            LOGW = maskp.tile([1, 512], F32)
            nc.vector.memset(LOGW, 0.0)

            with tc.tile_pool(name="psm", bufs=2, space="PSUM") as psm, \
                 tc.tile_pool(name="mtmp", bufs=2) as mtmp:
                rows_b = []
                for i in range(4):
                    tb = mtmp.tile([1, 512], F32, tag=f"rows{i}")
                    nc.sync.dma_start(out=tb, in_=rows[i:i + 1, :])
                    rows_b.append(tb)
                i2_i = mtmp.tile([128, 512], I32)
                nc.gpsimd.iota(i2_i, pattern=[[0, 512]], base=0, channel_multiplier=2)
                i2f = mtmp.tile([128, 512], F32, tag="i2f")
                nc.vector.tensor_copy(i2f, i2_i)
                pm = psm.tile([128, 512], F32, tag="pm")
                nc.tensor.matmul(pm, ones_row, rows_b[0], start=True, stop=True)
                sblf = mtmp.tile([128, 512], F32, tag="sblf")
                nc.vector.tensor_add(sblf, i2f, pm)
                pm2 = psm.tile([128, 512], F32, tag="pm")
                nc.tensor.matmul(pm2, ones_row, rows_b[1], start=True, stop=True)
                c1 = mtmp.tile([128, 512], F32, tag="c1")
                nc.vector.tensor_tensor(c1, sblf, pm2, OP.is_ge)
                pm3 = psm.tile([128, 512], F32, tag="pm")
                nc.tensor.matmul(pm3, ones_row, rows_b[2], start=True, stop=True)
                c2 = mtmp.tile([128, 512], F32, tag="c2")
                nc.vector.tensor_tensor(c2, sblf, pm3, OP.is_le)
                nc.vector.tensor_mul(mbl, c1, c2)
                pm4 = psm.tile([128, 512], F32, tag="pm4")
                nc.tensor.matmul(pm4, ones_row, rows_b[3], start=True, stop=True)
                sblf2 = mtmp.tile([128, 512], F32, tag="sblf2")
                nc.vector.tensor_scalar(sblf2, sblf, 2.0, None, OP.add)
                c1l = mtmp.tile([128, 512], F32, tag="c1l")
                nc.vector.tensor_tensor(c1l, sblf2, pm2, OP.is_ge)
                c2l = mtmp.tile([128, 512], F32, tag="c2l")
                nc.vector.tensor_tensor(c2l, sblf2, pm4, OP.is_le)
                nc.vector.tensor_mul(mla, c1l, c2l)
                initB = mtmp.tile([128, NB], F32, tag="ib")
                nc.vector.tensor_scalar(initB, sblf[:, 0:NB], 0.0, None, OP.is_equal)
                nc.vector.tensor_scalar(initBC, initB, CENTER, None, OP.mult)
                nc.vector.tensor_scalar(initLC, initB, CENTER, None, OP.mult)

            # ---------- probability tiles (128, TP, NB) ----------
            PL = bigp.tile([128, TP, NB], F32)
            QL = bigp.tile([128, TP, NB], F32)
            PB = bigp.tile([128, TP, NB], F32)

            with tc.tile_pool(name="psg", bufs=2, space="PSUM") as psg, \
                 tc.tile_pool(name="psT", bufs=2, space="PSUM") as psT, \
                 tc.tile_pool(name="grows", bufs=1) as grows, \
                 tc.tile_pool(name="stage", bufs=3) as stage, \
                 tc.tile_pool(name="ohp", bufs=2) as ohp:
                thr_b, tgt_b, lpbm_b = [], [], []
                for b in range(NB):
                    tb = grows.tile([1, T], F32, tag=f"thr{b}")
                    nc.sync.dma_start(out=tb, in_=thr[b:b + 1, :])
                    thr_b.append(tb)
                    tb = grows.tile([1, L], F32, tag=f"tgt{b}")
                    nc.sync.dma_start(out=tb, in_=tgtrot[b:b + 1, :])
                    tgt_b.append(tb)
                    tb = grows.tile([1, T], F32, tag=f"lpbm{b}")
                    nc.sync.dma_start(out=tb, in_=lpbm[b:b + 1, :])
                    lpbm_b.append(tb)
                skpS = grows.tile([L, NB], F32)
                nc.sync.dma_start(out=skpS, in_=skiprot[:, :])
                valS = grows.tile([L, NB], F32)
                nc.sync.dma_start(out=valS, in_=valrot[:, :])
                for b in range(NB):
                    pT = psT.tile([128, L], F32, tag="pT")
                    nc.tensor.matmul(pT, ones_row, tgt_b[b], start=True, stop=True)
                    ohs = []
                    for vc in range(4):
                        oh = ohp.tile([128, L], F32, tag=f"oh{vc}")
                        nc.vector.tensor_scalar(oh, pT, iota_k[vc], None, OP.is_equal,
                                                OP.bypass)
                        ohs.append(oh)
                    for tci in range(n_tc):
                        pg = psg.tile([128, TC], F32, tag="pg")
                        for vc in range(4):
                            st = stage.tile([128, TC], F32, tag="st")
                            nc.sync.dma_start(
                                out=st,
                                in_=lpT[b, 128 * vc:128 * (vc + 1),
                                        TC * tci:TC * (tci + 1)])
                            nc.tensor.matmul(pg, ohs[vc], st,
                                             start=(vc == 0), stop=False)
                        nc.tensor.matmul(pg, ones_row,
                                         thr_b[b][:, TC * tci:TC * (tci + 1)],
                                         start=False, stop=True)
                        nc.scalar.activation(PL[:, TC * tci:TC * (tci + 1), b], pg,
                                             AF.Exp)
                        pgb = psg.tile([128, TC], F32, tag="pg")
                        nc.tensor.matmul(pgb, ones_row,
                                         lpbm_b[b][:, TC * tci:TC * (tci + 1)],
                                         start=True, stop=True)
                        nc.scalar.activation(PB[:, TC * tci:TC * (tci + 1), b], pgb,
                                             AF.Exp)
                # absorbing pad step at t=1024: pl=0, pb=1 (ql=0 follows)
                nc.vector.memset(PL[:, T, :], 0.0)
                nc.vector.memset(PB[:, T, :], 1.0)
                for b in range(NB):
                    # zero label-prob rows with no label (padding slots) so
                    # they cannot grow at ~1/step and overflow mid-window
                    nc.vector.tensor_scalar(PL[:, :, b], PL[:, :, b],
                                            valS[:, b:b + 1], None, OP.mult)
                for b in range(NB):
                    nc.vector.tensor_scalar(QL[:, :, b], PL[:, :, b],
                                            skpS[:, b:b + 1], None, OP.mult)

            # ---------- bulk: 2-step coefficient tiles ----------
            # CLCB layout: (128, tau 512, fam 2, b 4, term 5); fam0=LA, fam1=BL
            CLCB = coefp.tile([128, NTAU, 2, NB, 5], F32)
            # odd/even prob slices: t = 2*tau+1 / 2*tau+2 for tau=0..511
            PL1 = PL[:, 1:TP:2, :]
            PL2 = PL[:, 2:TP:2, :]
            QL1 = QL[:, 1:TP:2, :]
            QL2 = QL[:, 2:TP:2, :]
            PB1 = PB[:, 1:TP:2, :]
            PB2 = PB[:, 2:TP:2, :]

            with tc.tile_pool(name="pssh", bufs=2, space="PSUM") as pssh, \
                 tc.tile_pool(name="btmp", bufs=1) as btmp:
                # partition-shifted odd-step probs: shPL1, shQL1 (128, 512, 4)
                shPL1 = btmp.tile([128, NTAU, NB], F32, tag="shPL1")
                shQL1 = btmp.tile([128, NTAU, NB], F32, tag="shQL1")
                for half in range(4):
                    sl = slice(128 * half, 128 * (half + 1))
                    ps = pssh.tile([128, 128, NB], F32, tag="ps")
                    nc.tensor.matmul(ps, SUB, PL1[:, sl, :], start=True, stop=True)
                    nc.vector.tensor_copy(shPL1[:, sl, :], ps)
                    ps2 = pssh.tile([128, 128, NB], F32, tag="ps")
                    nc.tensor.matmul(ps2, SUB, QL1[:, sl, :], start=True, stop=True)
                    nc.vector.tensor_copy(shQL1[:, sl, :], ps2)

                tmp1 = btmp.tile([128, NTAU, NB], F32, tag="tmp1")
                # c0 = PL2*PL1
                nc.vector.tensor_mul(CLCB[:, :, 0, :, 0], PL2, PL1)
                # c1 = PL2*(PL1+PB1)
                nc.vector.tensor_add(tmp1, PL1, PB1)
                nc.vector.tensor_mul(CLCB[:, :, 0, :, 1], PL2, tmp1)
                # c3 = QL2*shPL1 ; c2 = PL2*(QL1+PB1) + c3
                nc.vector.tensor_mul(CLCB[:, :, 0, :, 3], QL2, shPL1)
                nc.vector.tensor_add(tmp1, QL1, PB1)
                nc.vector.tensor_mul(tmp1, PL2, tmp1)
                nc.vector.tensor_add(CLCB[:, :, 0, :, 2], tmp1,
                                     CLCB[:, :, 0, :, 3])
                # c4 = QL2*shQL1
                nc.vector.tensor_mul(CLCB[:, :, 0, :, 4], QL2, shQL1)
                # d0 = 0 ; d1 = PB2*PB1
                nc.vector.memset(CLCB[:, :, 1, :, 0], 0.0)
                nc.vector.tensor_mul(CLCB[:, :, 1, :, 1], PB2, PB1)
                # d2 = PB2*(PB1+shPL1) ; d3 = PB2*shPL1 ; d4 = PB2*shQL1
                nc.vector.tensor_add(tmp1, PB1, shPL1)
                nc.vector.tensor_mul(CLCB[:, :, 1, :, 2], PB2, tmp1)
                nc.vector.tensor_mul(CLCB[:, :, 1, :, 3], PB2, shPL1)
                nc.vector.tensor_mul(CLCB[:, :, 1, :, 4], PB2, shQL1)

            # ---------- serial phase: 512 super-steps ----------
            with tc.tile_pool(name="pstep", bufs=2, space="PSUM") as pstep, \
                 tc.tile_pool(name="psrs", bufs=2, space="PSUM") as psrs, \
                 tc.tile_pool(name="work", bufs=3) as work:

                # state S: (128, fam 2, b 4); fam0=LA, fam1=BL
                S = work.tile([128, 2, NB], F32, tag="S")
                nc.vector.tensor_mul(S[:, 0, :], initLC, PL[:, 0, :])
                nc.vector.tensor_mul(S[:, 1, :], initBC, PB[:, 0, :])

                for tau in range(NTAU):
                    # OPS psum (128, term 5, b 4):
                    #   term0 = LA, 1 = BL, 2 = shLA, 3 = shBL, 4 = sh2LA
                    OPS = pstep.tile([128, 5, NB], F32, tag="OPS")
                    nc.tensor.matmul(OPS[:, 0:2, :], ID, S, start=True, stop=True)
                    nc.tensor.matmul(OPS[:, 2:4, :], SUB, S, start=True, stop=True)
                    nc.tensor.matmul(OPS[:, 4, :], SUB2, S[:, 0, :],
                                     start=True, stop=True)
                    # M = OPS (as [fam, b, term], bcast over fam) * CLCB[tau]
                    ops_v = (OPS[:, :, :].transpose([0, 2, 1]).unsqueeze(1)
                             .broadcast_to((128, 2, NB, 5)))
                    M = work.tile([128, 2, NB, 5], F32, tag="M")
                    nc.vector.tensor_tensor(M, ops_v, CLCB[:, tau, :, :, :],
                                            OP.mult)
                    Sn = work.tile([128, 2, NB], F32, tag="S")
                    nc.vector.tensor_reduce(Sn, M, mybir.AxisListType.X, OP.add)
                    S = Sn

                    if (tau + 1) % 8 == 0 and (tau + 1) // 8 <= n_rs:
                        ri = (tau + 1) // 8 - 1
                        Sm = work.tile([128, 2, NB], F32, tag="Sm")
                        nc.vector.tensor_tensor(Sm[:, 0, :], S[:, 0, :],
                                                mla[:, NB * ri:NB * (ri + 1)],
                                                OP.mult)
                        nc.vector.tensor_tensor(Sm[:, 1, :], S[:, 1, :],
                                                mbl[:, NB * ri:NB * (ri + 1)],
                                                OP.mult)
                        pS = psrs.tile([1, NB], F32, tag="psS")
                        nc.tensor.matmul(pS, ones_col, Sm[:, 1, :],
                                         start=True, stop=False)
                        nc.tensor.matmul(pS, ones_col, Sm[:, 0, :],
                                         start=False, stop=True)
                        wrec = work.tile([1, NB], F32, tag="wrec")
                        nc.vector.reciprocal(wrec, pS)
                        nc.scalar.copy(LOGW[0:1, NB * ri:NB * (ri + 1)], pS)
                        pR = psrs.tile([128, NB], F32, tag="psR")
                        nc.tensor.matmul(pR, ones_row, wrec, start=True, stop=True)
                        # two-stage: x*(1/sum) <= 1, then *CENTER — a fused
                        # CENTER/sum factor overflows f32 for low-prob windows
                        S2 = work.tile([128, 2, NB], F32, tag="S")
                        nc.vector.tensor_tensor(S2[:, 0, :], Sm[:, 0, :], pR,
                                                OP.mult)
                        nc.vector.tensor_tensor(S2[:, 1, :], Sm[:, 1, :], pR,
                                                OP.mult)
                        nc.vector.tensor_scalar(S2, S2, CENTER, None, OP.mult)
                        S = S2

                # ---------- output ----------
                nc.sync.dma_start(out=outw[0:1, :], in_=LOGW)
                nc.sync.dma_start(out=outd[1:2, :], in_=S[127:128, 1, :])

    nc.compile()
    return nc


def host_prep(log_probs, targets, input_lengths, target_lengths, T=T_FULL):
    """Build the per-core input maps (host-side sharding + index prep)."""
    log_probs = np.asarray(log_probs, np.float32)
    targets = np.asarray(targets).astype(np.int64)
    il = np.asarray(input_lengths).astype(np.int64)
    tl = np.asarray(target_lengths).astype(np.int64)
    in_maps = []
    t_ar = np.arange(T)
    for c in range(NCORES):
        s = slice(c * NB, (c + 1) * NB)
        lp = log_probs[s, :T]
        ilc, tlc = il[s], tl[s]
        tg = targets[s]
        lpT = np.ascontiguousarray(np.transpose(lp, (0, 2, 1)))      # (NB, V, T)
        lpb = np.ascontiguousarray(lp[:, :, 0])                      # (NB, T)
        absorb = t_ar[None, :] >= ilc[:, None]
        thr = np.where(absorb, np.float32(NEG), np.float32(0.0)).astype(np.float32)
        lpbm = np.where(absorb, np.float32(0.0), lpb).astype(np.float32)
        rot = 127 - tlc                                              # (NB,)
        tgtrot = np.full((NB, L), -1.0, np.float32)
        skiprot = np.zeros((L, NB), np.float32)
        for b in range(NB):
            r0 = rot[b]
            n = tlc[b]
            tgtrot[b, r0:r0 + n] = tg[b, :n].astype(np.float32)
            if n > 1:
                sk = (tg[b, 1:n] != tg[b, :n - 1]).astype(np.float32)
                skiprot[r0 + 1:r0 + n, b] = sk
        n_ri = 128
        ri = np.arange(n_ri)
        t_r = R * (ri + 1)
        lo = 2 * tlc[None, :] - 2 * np.maximum(ilc[None, :] - t_r[:, None], 0)
        hi = np.minimum(2 * t_r[:, None] + 1, 2 * tlc[None, :])
        soff = np.broadcast_to((2 * tlc - 254)[None, :], (n_ri, NB))
        rows = np.zeros((4, 512), np.float32)
        rows[0, :] = soff.reshape(-1)[:512]
        rows[1, :] = lo.reshape(-1)[:512]
        rows[2, :] = hi.reshape(-1)[:512]
        rows[3, :] = (hi + 1).reshape(-1)[:512]
        valrot = np.ascontiguousarray((tgtrot >= 0).astype(np.float32).T)
        in_maps.append({
            "lpT": lpT, "lpbm": lpbm, "thr": thr,
            "tgtrot": tgtrot, "skiprot": skiprot, "valrot": valrot,
            "rows": rows,
        })
    return in_maps


_NC_CACHE = {}


def _get_nc(T=T_FULL):
    if T not in _NC_CACHE:
        _NC_CACHE[T] = build_nc()
    return _NC_CACHE[T]


def finish(results, target_lengths, T=T_FULL):
    tl = np.asarray(target_lengths).astype(np.int64)
    n_rs = (T - 1) // R
    pers = []
    for c in range(NCORES):
        out = results[c]["out"]
        fin = out[1].astype(np.float64)
        wv = results[c]["logw"][0].astype(np.float64).reshape(128, NB)[:n_rs]
        logs = np.log(np.maximum(wv, 1e-300)).sum(0)
        tlc = tl[c * NB:(c + 1) * NB].astype(np.float64)
        ll = np.log(np.maximum(fin, 1e-300)) + logs - (n_rs + 1) * np.log(CENTER)
        per = -ll / tlc
        pers.append(per)
    return np.float32(np.mean(np.concatenate(pers)))


def kernel(log_probs, targets, input_lengths, target_lengths):
    nc = _get_nc()
    in_maps = host_prep(log_probs, targets, input_lengths, target_lengths)
    res = run_bass_kernel_spmd(nc, in_maps, core_ids=list(range(NCORES)))
    return finish(res.results, target_lengths)
